# revision 1
# baseline (speedup 1.0000x reference)
"""Causal self-attention (D=1024, H=16, S=2048, B=2) on 8 trn2 cores.

Sharding: core i handles batch b = i // 4 and head-group g = i % 4
(4 heads = 256 model dims per group). Each core computes
    y_partial[b,g] = softmax_causal(Q K^T / 8) V  @ Wo[rows of g]
for its 4 heads; the host sums the 4 group partials per batch and adds bo.

Per-core kernel (bf16 matmul operands, fp32 PSUM accumulation):
  phase 0: xT[c] <- DMA-transpose of x columns (bf16 XBAR path)
  phase 1: QT/KT = (Wq/Wk)^T x^T + b (head pairs packed on partitions),
           V [t,d] for 4 heads + a ones column (softmax denominator trick)
  phase 2: per (head, 512-col s-block): scoresT = KT^T QT with causal block
           skipping, exp on ACT (2-tile batches), triangular diag-chunk mask
           via gpsimd affine_select, PV accumulation -> outT [65, s] whose
           row 64 is the denominator; reciprocal + DRAM-bounce broadcast +
           DVE multiply -> normalized A^T, packed into head-pair tiles
  phase 3: y = A Wo with K=128 head-pair accumulation
"""

import sys

sys.path.insert(0, "/opt/trn_rl_repo")

import ml_dtypes
import numpy as np

import concourse.bass as bass
import concourse.mybir as mybir
import concourse.tile as tile
from concourse import bacc

P = 128
S = 2048
D = 1024
NH = 4                    # heads per core
DH = 64                   # head dim
DPC = NH * DH             # model dims per core = 256
N_CT = D // P             # 8 contraction chunks
N_ST = S // P             # 16 t tiles of 128
N_SB = S // 512           # 4 s blocks of 512
F32 = mybir.dt.float32
BF16 = mybir.dt.bfloat16
SCALE = 1.0 / 8.0         # 1/sqrt(64)

AF = mybir.ActivationFunctionType
ALU = mybir.AluOpType


def build_nc(mm_mode: str = "bf16", stop_after: int = 99,
             skip_norm: bool = False) -> bass.Bass:
    nc = _build(mm_mode, stop_after, skip_norm)
    if not nc.is_finalized():
        nc.finalize()
    return nc


def _build(mm_mode: str, stop_after: int, skip_norm: bool) -> bass.Bass:
    assert mm_mode == "bf16"
    nc = bacc.Bacc("TRN2", target_bir_lowering=False, debug=False,
                   num_devices=8)

    x_d = nc.dram_tensor("x", [S, D], BF16, kind="ExternalInput")
    wq_d = nc.dram_tensor("wq", [D, DPC], BF16, kind="ExternalInput")
    wk_d = nc.dram_tensor("wk", [D, DPC], BF16, kind="ExternalInput")
    wv_d = nc.dram_tensor("wv", [D, DPC], BF16, kind="ExternalInput")
    wo_d = nc.dram_tensor("wo", [DPC, D], BF16, kind="ExternalInput")
    bq_d = nc.dram_tensor("bq", [DPC], F32, kind="ExternalInput")
    bk_d = nc.dram_tensor("bk", [DPC], F32, kind="ExternalInput")
    bv_d = nc.dram_tensor("bv", [DPC], F32, kind="ExternalInput")
    y_d = nc.dram_tensor("y", [S, D], F32, kind="ExternalOutput")

    with tile.TileContext(nc) as tc:
        with (
            tc.tile_pool(name="const", bufs=1) as const,
            tc.tile_pool(name="xtp", bufs=1) as xtp,
            tc.tile_pool(name="qkv", bufs=1) as qkv,
            tc.tile_pool(name="atp", bufs=1) as atp,
            tc.tile_pool(name="work", bufs=5) as work,
            tc.tile_pool(name="att", bufs=4) as attw,
            tc.tile_pool(name="denp", bufs=4) as denp,
            tc.tile_pool(name="rbp", bufs=4) as rbp,
            tc.tile_pool(name="dscr", bufs=8, space="DRAM") as dscr,
            tc.tile_pool(name="ps", bufs=2, space="PSUM") as psp,
            tc.tile_pool(name="ppv", bufs=4, space="PSUM") as ppv,
        ):
            # ---- weights / constants ----
            wq_s = const.tile([P, N_CT, DPC], BF16)
            wk_s = const.tile([P, N_CT, DPC], BF16)
            wv_s = const.tile([P, N_CT, DPC], BF16)
            nc.sync.dma_start(wq_s, wq_d.rearrange("(o p) d -> p o d", p=P))
            nc.sync.dma_start(wk_s, wk_d.rearrange("(o p) d -> p o d", p=P))
            nc.sync.dma_start(wv_s, wv_d.rearrange("(o p) d -> p o d", p=P))
            # Wo packed by head pairs: rows 128*dc .. 128*dc+127
            wo_s = const.tile([P, 2, D], BF16)
            nc.sync.dma_start(wo_s, wo_d.rearrange("(dc p) e -> p dc e", p=P))

            bq_s = const.tile([P, 2], F32)
            bk_s = const.tile([P, 2], F32)
            nc.sync.dma_start(bq_s, bq_d.rearrange("(o p) -> p o", p=P))
            nc.sync.dma_start(bk_s, bk_d.rearrange("(o p) -> p o", p=P))
            bv_b = const.tile([P, DPC], F32)
            nc.gpsimd.dma_start(
                out=bv_b, in_=bv_d[:].unsqueeze(0).partition_broadcast(P)
            )

            # ---- phase 0: DMA-transpose x into per-chunk xT tiles ----
            # s-block-major so phase-1 groups for early s-blocks can start
            # while later transposes are still on the XBAR.
            xT = [xtp.tile([P, S], BF16, tag=f"xt{c}", name=f"xt{c}")
                  for c in range(N_CT)]
            for g in range(N_SB):
                for c in range(N_CT):
                    nc.sync.dma_start_transpose(
                        xT[c][:, g * 512:(g + 1) * 512],
                        x_d[g * 512:(g + 1) * 512, c * P:(c + 1) * P])

            if stop_after <= 0:
                ys0 = work.tile([P, D], F32, tag="work")
                nc.vector.tensor_copy(ys0, xT[0][:, 0:1024])
                nc.sync.dma_start(y_d[0:P, :], ys0)
                return nc

            # ---- phase 1: projections ----
            # QT/KT: [128 (head-pair d), dc, s]
            QT = qkv.tile([P, 2, S], BF16)
            KT = qkv.tile([P, 2, S], BF16)
            # V_aug: [t-part, t-chunk, head, 65], col 64 == 1.0
            vaug = qkv.tile([P, N_ST, NH, DH + 1], BF16)
            nc.vector.memset(vaug[:, :, :, DH:DH + 1], 1.0)
            vaug_v = vaug[:, :, :, 0:DH]

            for sb in range(N_SB):
                for dc in range(2):
                    for w_s, b_s, dst in ((wq_s, bq_s, QT), (wk_s, bk_s, KT)):
                        ps = psp.tile([P, 1024], F32, tag="ps")
                        for c in range(N_CT):
                            nc.tensor.matmul(
                                ps[:, 0:512],
                                w_s[:, c, dc * P:(dc + 1) * P],
                                xT[c][:, sb * 512:(sb + 1) * 512],
                                start=(c == 0),
                                stop=(c == N_CT - 1),
                            )
                        # psum -> sbuf (bf16) with per-partition bias add
                        nc.vector.tensor_scalar_add(
                            dst[:, dc, sb * 512:(sb + 1) * 512],
                            ps[:, 0:512],
                            b_s[:, dc:dc + 1],
                        )

            for tt in range(N_ST):
                ps = psp.tile([P, 1024], F32, tag="ps")
                pvs = ps[:, 0:DPC]
                for c in range(N_CT):
                    nc.tensor.matmul(
                        pvs,
                        xT[c][:, tt * P:(tt + 1) * P],
                        wv_s[:, c, :],
                        start=(c == 0),
                        stop=(c == N_CT - 1),
                    )
                nc.vector.tensor_add(
                    vaug_v[:, tt, :, :],
                    pvs.rearrange("p (h u) -> p h u", h=NH),
                    bv_b.rearrange("p (h u) -> p h u", h=NH),
                )

            if stop_after <= 1:
                ys0 = work.tile([P, D], F32, tag="work")
                nc.vector.tensor_copy(ys0, QT[:, 0, 0:1024])
                nc.sync.dma_start(y_d[0:P, :], ys0)
                ys1 = work.tile([P, 3, DH], F32, tag="work")
                nc.vector.tensor_copy(ys1, vaug[:, 0:3, 0, 0:DH])
                nc.sync.dma_start(y_d[P:2 * P, 0:192], ys1)
                return nc

            # AT packed by head pairs: [128, dc, s]
            AT = atp.tile([P, 2, S], BF16)

            # ---- phase 2: attention ----
            # Head pairs (2*dc, 2*dc+1) share each score/exp tile: the two
            # K=64 score matmuls go to PE row-groups 0 and 64 (concurrent).
            for dc in range(2):
                for sb in range(N_SB):
                    pvs2 = [ppv.tile([DH + 1, 512], F32, tag="pv",
                                     name=f"pv{dc}_{sb}_{e}")
                            for e in range(2)]
                    t_cnt = 4 * sb + 4
                    for T in range(t_cnt):
                        k = T - 4 * sb
                        ms = 128 * k if k > 0 else 0
                        sc = psp.tile([P, 2, 512], F32, tag="ps")
                        ex = attw.tile([P, 2, 512], BF16, tag="ex")
                        for e in range(2):  # even/odd head of the pair
                            off = DH * e
                            nc.tensor.matmul(
                                sc[:, e, ms:512],
                                KT[off:off + DH, dc, T * P:(T + 1) * P],
                                QT[off:off + DH, dc,
                                   sb * 512 + ms:(sb + 1) * 512],
                                start=True,
                                stop=True,
                            )
                        nc.scalar.activation(
                            ex[:, :, ms:512], sc[:, :, ms:512],
                            AF.Exp, scale=SCALE,
                        )
                        if k >= 0:  # triangular mask on diagonal chunks
                            nc.gpsimd.affine_select(
                                out=ex[:, :, ms:ms + P],
                                in_=ex[:, :, ms:ms + P],
                                compare_op=ALU.is_ge,
                                fill=0.0,
                                base=0,
                                pattern=[[0, 2], [1, P]],
                                channel_multiplier=-1,
                            )
                        for e in range(2):
                            h = 2 * dc + e
                            nc.tensor.matmul(
                                pvs2[e][:, ms:512],
                                vaug[:, T, h, :],
                                ex[:, e, ms:512],
                                start=(T == 0),
                                stop=(T == t_cnt - 1),
                            )
                    for e in range(2):
                        pv = pvs2[e]
                        if skip_norm:
                            if e == 0:
                                nc.vector.tensor_copy(
                                    AT[0:DH, dc, sb * 512:(sb + 1) * 512],
                                    pv[0:DH, :])
                            continue
                        # normalize: row 64 of pv is the denominator
                        den = denp.tile([DH + 1, 512], F32, name="den")
                        nc.vector.reciprocal(
                            out=den[DH:DH + 1, :], in_=pv[DH:DH + 1, :]
                        )
                        dend = dscr.tile([512], F32, name="dend")
                        nc.gpsimd.dma_start(out=dend, in_=den[DH:DH + 1, :])
                        rb = rbp.tile([DH, 512], F32)
                        nc.gpsimd.dma_start(
                            out=rb,
                            in_=dend[:].unsqueeze(0).partition_broadcast(DH),
                        )
                        if e == 0:
                            nc.vector.tensor_mul(
                                AT[0:DH, dc, sb * 512:(sb + 1) * 512],
                                pv[0:DH, :], rb)
                        else:
                            att = attw.tile([DH, 512], BF16, tag="att")
                            nc.vector.tensor_mul(att, pv[0:DH, :], rb)
                            nc.sync.dma_start(
                                AT[DH:P, dc, sb * 512:(sb + 1) * 512], att)

            if stop_after <= 2:
                ys0 = work.tile([DH, S], F32, tag="work")
                nc.vector.tensor_copy(ys0, AT[0:DH, 0, :])
                nc.sync.dma_start(y_d[0:DH, 0:1024], ys0[:, 0:1024])
                nc.sync.dma_start(y_d[DH:2 * DH, 0:1024], ys0[:, 1024:2048])
                return nc

            # ---- phase 3: output projection (K=128 head pairs) ----
            for st in range(N_ST):
                for eb in range(2):
                    ps = psp.tile([P, 1024], F32, tag="ps")
                    for dc in range(2):
                        nc.tensor.matmul(
                            ps[:, 0:512],
                            AT[:, dc, st * P:(st + 1) * P],
                            wo_s[:, dc, eb * 512:(eb + 1) * 512],
                            start=(dc == 0),
                            stop=(dc == 1),
                        )
                    ys = work.tile([P, 512], F32, tag="work")
                    nc.scalar.copy(ys, ps[:, 0:512])
                    nc.sync.dma_start(
                        y_d[st * P:(st + 1) * P, eb * 512:(eb + 1) * 512], ys
                    )

    return nc


_NC_CACHE = {}


def _get_nc(mm_mode="bf16"):
    if mm_mode not in _NC_CACHE:
        _NC_CACHE[mm_mode] = build_nc(mm_mode=mm_mode)
    return _NC_CACHE[mm_mode]


MM_MODE = "bf16"


def make_in_maps(x, Wq, bq, Wk, bk, Wv, bv, Wo, mm_mode=None):
    """Per-core input dicts: core i -> (batch i//4, head-group i%4)."""
    bf = ml_dtypes.bfloat16
    in_maps = []
    for core in range(8):
        b, g = core // 4, core % 4
        sl = slice(g * DPC, (g + 1) * DPC)
        in_maps.append({
            "x": np.ascontiguousarray(x[b]).astype(bf),
            "wq": np.ascontiguousarray(Wq[:, sl]).astype(bf),
            "wk": np.ascontiguousarray(Wk[:, sl]).astype(bf),
            "wv": np.ascontiguousarray(Wv[:, sl]).astype(bf),
            "wo": np.ascontiguousarray(Wo[sl, :]).astype(bf),
            "bq": np.ascontiguousarray(bq[sl]).astype(np.float32),
            "bk": np.ascontiguousarray(bk[sl]).astype(np.float32),
            "bv": np.ascontiguousarray(bv[sl]).astype(np.float32),
        })
    return in_maps


def combine_results(results, bo):
    out = np.zeros((2, S, D), dtype=np.float32)
    for core in range(8):
        out[core // 4] += results[core]["y"]
    out += bo.astype(np.float32)
    return out


_RUNNER_CACHE = {}


def get_runner(mm_mode=None):
    """Build (once) a jitted 8-core runner; returns fn(in_maps) -> results."""
    mode = mm_mode or MM_MODE
    if mode in _RUNNER_CACHE:
        return _RUNNER_CACHE[mode]

    import jax
    from jax.sharding import Mesh, PartitionSpec
    from jax.experimental.shard_map import shard_map
    from concourse import bass2jax, mybir as _mb

    nc = _get_nc(mode)
    bass2jax.install_neuronx_cc_hook()

    pname = nc.partition_id_tensor.name if nc.partition_id_tensor else None
    in_names, out_names, out_avals = [], [], []
    for alloc in nc.m.functions[0].allocations:
        if not isinstance(alloc, _mb.MemoryLocationSet):
            continue
        name = alloc.memorylocations[0].name
        if alloc.kind == "ExternalInput":
            if name != pname:
                in_names.append(name)
        elif alloc.kind == "ExternalOutput":
            out_names.append(name)
            out_avals.append(jax.core.ShapedArray(
                tuple(alloc.tensor_shape), _mb.dt.np(alloc.dtype)))
    n_params = len(in_names)
    all_names = in_names + out_names
    if pname is not None:
        all_names = all_names + [pname]

    def _body(*args):
        operands = list(args)
        if pname is not None:
            operands.append(bass2jax.partition_id_tensor())
        outs = bass2jax._bass_exec_p.bind(
            *operands,
            out_avals=tuple(out_avals),
            in_names=tuple(all_names),
            out_names=tuple(out_names),
            lowering_input_output_aliases=(),
            sim_require_finite=True,
            sim_require_nnan=True,
            nc=nc,
        )
        return tuple(outs)

    devices = jax.devices()[:8]
    mesh = Mesh(np.asarray(devices), ("core",))
    sharded = jax.jit(
        shard_map(_body, mesh=mesh,
                  in_specs=(PartitionSpec("core"),) * (n_params + len(out_names)),
                  out_specs=(PartitionSpec("core"),) * len(out_names),
                  check_rep=False),
        keep_unused=True,
    )

    from jax.sharding import NamedSharding
    zero_outs = [
        jax.device_put(
            np.zeros((8 * a.shape[0], *a.shape[1:]), a.dtype),
            NamedSharding(mesh, PartitionSpec("core")),
        )
        for a in out_avals
    ]

    def run(in_maps):
        concat_in = [
            np.concatenate([np.asarray(m[name]) for m in in_maps], axis=0)
            for name in in_names
        ]
        out_arrs = sharded(*concat_in, *zero_outs)
        return [
            {name: np.asarray(out_arrs[i]).reshape(8, *out_avals[i].shape)[c]
             for i, name in enumerate(out_names)}
            for c in range(8)
        ]

    run.sharded = sharded
    run.in_names = in_names
    run.out_names = out_names
    run.out_avals = out_avals
    run.zero_outs = zero_outs
    _RUNNER_CACHE[mode] = run
    return run


def kernel(x, Wq, bq, Wk, bk, Wv, bv, Wo, bo, **_ignored):
    x = np.asarray(x, dtype=np.float32)
    in_maps = make_in_maps(
        x,
        np.asarray(Wq, np.float32), np.asarray(bq, np.float32),
        np.asarray(Wk, np.float32), np.asarray(bk, np.float32),
        np.asarray(Wv, np.float32), np.asarray(bv, np.float32),
        np.asarray(Wo, np.float32),
    )
    try:
        results = get_runner(MM_MODE)(in_maps)
    except Exception:
        # fallback: stock SPMD runner (slower dispatch, same NEFF)
        from concourse.bass_utils import run_bass_kernel_spmd
        results = run_bass_kernel_spmd(
            _get_nc(MM_MODE), in_maps, core_ids=list(range(8))).results
    return combine_results(results, np.asarray(bo, np.float32))



# revision 2
# speedup vs baseline: 38.4410x; 38.4410x over previous
"""Causal self-attention (D=1024, H=16, S=2048, B=2) — dispatch-lean rewrite.

Design notes (vs the staged baseline kernel.py):
- The dominant cost on this axon-tunneled setup is per-exec dispatch work
  proportional to (#operands x #cores), not device compute. So inputs are
  packed into ONE bf16 blob per core (pre-transposed x + weight slices in
  exactly the SBUF layouts the kernel wants) + one small f32 bias tensor.
- GPC = head-group-pairs per core; n_cores = 8 // GPC. Each core processes
  GPC (batch, head-group) pairs sequentially; 4 head-groups of 4 heads each
  per batch. GPC=1 reproduces the baseline sharding with 3 operands.
- Softmax denominator broadcast uses gpsimd partition_broadcast (SBUF->SBUF)
  instead of the baseline's DRAM bounce.
- Runner compiled via bass2jax.fast_dispatch_compile (C++ fast-path dispatch).

Blob layout per core ([128, COLS] bf16):
  for each local batch lb: xT region [128, 8, 2048]  (xT[p,c,s] = x[b,s,128c+p])
  then per local group gi:
    wqkv [128, 3, 8, 256]  (w[p,j,o,d] = W_j[128o+p, 256g+d], j in q,k,v)
    wo   [128, 2, 1024]    (wo[p,dc,e] = Wo[256g+128dc+p, e])
Bias tensor [GPC, 3, 256] f32 (bq, bk, bv slices per local group).
Output y [n_b*2048, 1024] f32; host adds bo (and sums partials when GPC<4).
"""

import sys

sys.path.insert(0, "/opt/trn_rl_repo")

import ml_dtypes
import numpy as np

import concourse.bass as bass
import concourse.mybir as mybir
import concourse.tile as tile
from concourse import bacc

P = 128
S = 2048
D = 1024
NH = 4                    # heads per group
DH = 64                   # head dim
DPC = NH * DH             # model dims per group = 256
N_CT = D // P             # 8 contraction chunks
N_ST = S // P             # 16 t tiles of 128
N_SB = S // 512           # 4 s blocks of 512
F32 = mybir.dt.float32
BF16 = mybir.dt.bfloat16
SCALE = 1.0 / 8.0         # 1/sqrt(64)
XCOLS = N_CT * S          # 16384 cols per batch xT region
BIAS_COLS = 512           # [bq 2][bk 2][bv 256][pad] bf16, bv replicated per row
WCOLS = 3 * N_CT * DPC + 2 * D + BIAS_COLS   # 8704 cols per group

AF = mybir.ActivationFunctionType
ALU = mybir.AluOpType

ALL_PAIRS = [(b, g) for b in range(2) for g in range(4)]


def core_pairs(gpc: int, core: int):
    return ALL_PAIRS[core * gpc:(core + 1) * gpc]


def build_nc(gpc: int) -> bass.Bass:
    """One NEFF shared by all cores of the gpc config (SPMD, identical
    structure; only the data differs)."""
    assert gpc in (1, 2, 4, 8)
    pairs = core_pairs(gpc, 0)
    local_batches = sorted({b for b, _ in pairs})
    n_b = len(local_batches)

    nc = bacc.Bacc("TRN2", target_bir_lowering=False, debug=False,
                   num_devices=8 // gpc, enable_partition_id=False)

    blob_d = nc.dram_tensor("blob", [P, n_b * XCOLS + gpc * WCOLS], BF16,
                            kind="ExternalInput")
    y_d = nc.dram_tensor("y", [n_b * S, D], F32, kind="ExternalOutput")
    wbase0 = n_b * XCOLS

    with tile.TileContext(nc) as tc:
        with (
            tc.tile_pool(name="const", bufs=2) as const,
            tc.tile_pool(name="xtp", bufs=1) as xtp,
            tc.tile_pool(name="wp", bufs=2) as wp,
            tc.tile_pool(name="wop", bufs=min(gpc, 4) + 1) as wop,
            tc.tile_pool(name="qkv", bufs=2) as qkv,
            tc.tile_pool(name="atp", bufs=min(gpc, 4)) as atp,
            tc.tile_pool(name="work", bufs=3) as work,
            tc.tile_pool(name="att", bufs=4) as attw,
            tc.tile_pool(name="denp", bufs=4) as denp,
            tc.tile_pool(name="rbp", bufs=4) as rbp,
            tc.tile_pool(name="ps", bufs=2, space="PSUM") as psp,
            tc.tile_pool(name="ppv", bufs=4, space="PSUM") as ppv,
        ):
            gi = 0
            for lb, b in enumerate(local_batches):
                # ---- xT for this batch: one 4MB DMA ----
                xT = xtp.tile([P, N_CT, S], BF16, tag="xt", name=f"xt{lb}")
                nc.sync.dma_start(
                    xT, blob_d[:, lb * XCOLS:(lb + 1) * XCOLS]
                    .rearrange("p (c s) -> p c s", c=N_CT))

                b_groups = [i for i, (bb, _) in enumerate(pairs) if bb == b]
                ATs = {}
                wos = {}
                for gi in b_groups:
                    wb = wbase0 + gi * WCOLS
                    w = wp.tile([P, 3, N_CT, DPC], BF16, tag="w", name=f"w{gi}")
                    nc.sync.dma_start(
                        w, blob_d[:, wb:wb + 3 * N_CT * DPC]
                        .rearrange("p (j o d) -> p j o d", j=3, o=N_CT))
                    wo_s = wop.tile([P, 2, D], BF16, tag="wo", name=f"wo{gi}")
                    wo0 = wb + 3 * N_CT * DPC
                    nc.sync.dma_start(
                        wo_s, blob_d[:, wo0:wo0 + 2 * D]
                        .rearrange("p (dc e) -> p dc e", dc=2))
                    wos[gi] = wo_s

                    # biases for this group: bf16 in blob -> f32 on DVE
                    bb = const.tile([P, 260], BF16, tag="bb")
                    nc.sync.dma_start(bb, blob_d[:, wo0 + 2 * D:
                                                 wo0 + 2 * D + 260])
                    bf_t = const.tile([P, 260], F32, tag="bf")
                    nc.vector.tensor_copy(bf_t, bb)
                    bqk_sc = bf_t[:, 0:4].rearrange("p (j o) -> p j o", j=2)
                    bv_v = bf_t[:, 4:260]

                    # ---- phase 1: projections ----
                    QT = qkv.tile([P, 2, S], BF16, tag="qt", name=f"qt{gi}")
                    KT = qkv.tile([P, 2, S], BF16, tag="kt", name=f"kt{gi}")
                    vaug = qkv.tile([P, N_ST, NH, DH + 1], BF16,
                                    tag="va", name=f"va{gi}")
                    nc.vector.memset(vaug[:, :, :, DH:DH + 1], 1.0)
                    vaug_v = vaug[:, :, :, 0:DH]

                    for sb in range(N_SB):
                        for dc in range(2):
                            for j, dst in ((0, QT), (1, KT)):
                                ps = psp.tile([P, 2, 512], F32, tag="ps")
                                for c in range(N_CT):
                                    nc.tensor.matmul(
                                        ps[:, 0, :],
                                        w[:, j, c, dc * P:(dc + 1) * P],
                                        xT[:, c, sb * 512:(sb + 1) * 512],
                                        start=(c == 0),
                                        stop=(c == N_CT - 1),
                                    )
                                nc.vector.tensor_scalar_add(
                                    dst[:, dc, sb * 512:(sb + 1) * 512],
                                    ps[:, 0, :],
                                    bqk_sc[:, j, dc:dc + 1],
                                )

                    for tt in range(N_ST):
                        ps = psp.tile([P, 2, 512], F32, tag="ps")
                        pvs = ps[:, 0, 0:DPC]
                        for c in range(N_CT):
                            nc.tensor.matmul(
                                pvs,
                                xT[:, c, tt * P:(tt + 1) * P],
                                w[:, 2, c, :],
                                start=(c == 0),
                                stop=(c == N_CT - 1),
                            )
                        nc.vector.tensor_add(
                            vaug_v[:, tt, :, :],
                            pvs.rearrange("p (h u) -> p h u", h=NH),
                            bv_v.rearrange("p (h u) -> p h u", h=NH),
                        )

                    # ---- phase 2: attention ----
                    AT = atp.tile([P, 2, S], BF16, tag="at", name=f"at{gi}")
                    ATs[gi] = AT
                    for dc in range(2):
                        for sb in range(N_SB):
                            pvs2 = [ppv.tile([DH + 1, 512], F32, tag="pv",
                                             name=f"pv{gi}_{dc}_{sb}_{e}")
                                    for e in range(2)]
                            t_cnt = 4 * sb + 4
                            for T in range(t_cnt):
                                k = T - 4 * sb
                                ms = 128 * k if k > 0 else 0
                                sc = psp.tile([P, 2, 512], F32, tag="ps")
                                ex = attw.tile([P, 2, 512], BF16, tag="ex")
                                for e in range(2):
                                    off = DH * e
                                    nc.tensor.matmul(
                                        sc[:, e, ms:512],
                                        KT[off:off + DH, dc, T * P:(T + 1) * P],
                                        QT[off:off + DH, dc,
                                           sb * 512 + ms:(sb + 1) * 512],
                                        start=True,
                                        stop=True,
                                    )
                                nc.scalar.activation(
                                    ex[:, :, ms:512], sc[:, :, ms:512],
                                    AF.Exp, scale=SCALE,
                                )
                                if k >= 0:
                                    nc.gpsimd.affine_select(
                                        out=ex[:, :, ms:ms + P],
                                        in_=ex[:, :, ms:ms + P],
                                        compare_op=ALU.is_ge,
                                        fill=0.0,
                                        base=0,
                                        pattern=[[0, 2], [1, P]],
                                        channel_multiplier=-1,
                                    )
                                for e in range(2):
                                    h = 2 * dc + e
                                    nc.tensor.matmul(
                                        pvs2[e][:, ms:512],
                                        vaug[:, T, h, :],
                                        ex[:, e, ms:512],
                                        start=(T == 0),
                                        stop=(T == t_cnt - 1),
                                    )
                            for e in range(2):
                                pv = pvs2[e]
                                # row 64 of pv is the softmax denominator
                                den = denp.tile([1, 512], F32, name="den")
                                nc.vector.reciprocal(
                                    out=den, in_=pv[DH:DH + 1, :])
                                rb = rbp.tile([DH, 512], F32)
                                nc.gpsimd.partition_broadcast(rb, den[0:1, :])
                                if e == 0:
                                    nc.vector.tensor_mul(
                                        AT[0:DH, dc, sb * 512:(sb + 1) * 512],
                                        pv[0:DH, :], rb)
                                else:
                                    att = attw.tile([DH, 512], BF16, tag="att")
                                    nc.vector.tensor_mul(att, pv[0:DH, :], rb)
                                    nc.sync.dma_start(
                                        AT[DH:P, dc, sb * 512:(sb + 1) * 512],
                                        att)

                # ---- phase 3: output projection for this batch ----
                kparts = [(gi, dc) for gi in b_groups for dc in range(2)]
                for st in range(N_ST):
                    ps = psp.tile([P, 2, 512], F32, tag="ps")
                    for eb in range(2):
                        for ki, (gi2, dc) in enumerate(kparts):
                            nc.tensor.matmul(
                                ps[:, eb, :],
                                ATs[gi2][:, dc, st * P:(st + 1) * P],
                                wos[gi2][:, dc, eb * 512:(eb + 1) * 512],
                                start=(ki == 0),
                                stop=(ki == len(kparts) - 1),
                            )
                    ys = work.tile([P, D], F32, tag="work")
                    nc.scalar.copy(ys[:, 0:512], ps[:, 0, :])
                    nc.scalar.copy(ys[:, 512:1024], ps[:, 1, :])
                    nc.sync.dma_start(
                        y_d[lb * S + st * P:lb * S + (st + 1) * P, :], ys)

    nc.finalize()
    return nc


_NC_CACHE = {}


def get_nc(gpc: int):
    if gpc not in _NC_CACHE:
        _NC_CACHE[gpc] = build_nc(gpc)
    return _NC_CACHE[gpc]


def make_in_maps(x, Wq, bq, Wk, bk, Wv, bv, Wo, gpc: int):
    """Per-core packed input dicts for the gpc config."""
    bf = ml_dtypes.bfloat16
    n_cores = 8 // gpc
    xT = {}  # b -> [128, 8*2048] bf16
    for b in range(2):
        t = np.ascontiguousarray(x[b].T)          # [1024, 2048]
        t = t.reshape(N_CT, P, S).transpose(1, 0, 2).reshape(P, XCOLS)
        xT[b] = t.astype(bf)

    def wslice(W, g):  # [128, 8, 256] -> [128, 8*256]
        return (W[:, g * DPC:(g + 1) * DPC]
                .reshape(N_CT, P, DPC).transpose(1, 0, 2).reshape(P, -1))

    def woslice(W, g):  # [128, 2, 1024] -> [128, 2*1024]
        return (W[g * DPC:(g + 1) * DPC, :]
                .reshape(2, P, D).transpose(1, 0, 2).reshape(P, -1))

    def biasblock(g):  # [128, 512]: [bq 2][bk 2][bv 256][pad]
        blk = np.zeros((P, BIAS_COLS), np.float32)
        blk[:, 0:2] = bq[g * DPC:(g + 1) * DPC].reshape(2, P).T
        blk[:, 2:4] = bk[g * DPC:(g + 1) * DPC].reshape(2, P).T
        blk[:, 4:260] = bv[g * DPC:(g + 1) * DPC][None, :]
        return blk

    in_maps = []
    for core in range(n_cores):
        pairs = core_pairs(gpc, core)
        local_batches = sorted({b for b, _ in pairs})
        pieces = [xT[b] for b in local_batches]
        for (b, g) in pairs:
            pieces += [
                wslice(Wq, g).astype(bf), wslice(Wk, g).astype(bf),
                wslice(Wv, g).astype(bf), woslice(Wo, g).astype(bf),
                biasblock(g).astype(bf),
            ]
        blob = np.ascontiguousarray(np.concatenate(pieces, axis=1))
        in_maps.append({"blob": blob})
    return in_maps


def combine_results(results, bo, gpc: int):
    out = np.zeros((2, S, D), dtype=np.float32)
    n_cores = 8 // gpc
    for core in range(n_cores):
        pairs = core_pairs(gpc, core)
        local_batches = sorted({b for b, _ in pairs})
        y = results[core]["y"].reshape(len(local_batches), S, D)
        for lb, b in enumerate(local_batches):
            out[b] += y[lb]
    out += bo.astype(np.float32)
    return out


_RUNNER_CACHE = {}


def get_runner(gpc: int):
    """Fast-dispatch jitted runner over 8//gpc cores."""
    if gpc in _RUNNER_CACHE:
        return _RUNNER_CACHE[gpc]

    import jax
    from jax.sharding import Mesh, PartitionSpec, NamedSharding
    from jax.experimental.shard_map import shard_map
    from concourse import bass2jax, mybir as _mb

    nc = get_nc(gpc)
    bass2jax.install_neuronx_cc_hook()
    n_cores = 8 // gpc

    pname = nc.partition_id_tensor.name if nc.partition_id_tensor else None
    in_names, out_names, out_avals = [], [], []
    for alloc in nc.m.functions[0].allocations:
        if not isinstance(alloc, _mb.MemoryLocationSet):
            continue
        name = alloc.memorylocations[0].name
        if alloc.kind == "ExternalInput":
            if name != pname:
                in_names.append(name)
        elif alloc.kind == "ExternalOutput":
            out_names.append(name)
            out_avals.append(jax.core.ShapedArray(
                tuple(alloc.tensor_shape), _mb.dt.np(alloc.dtype)))
    n_params = len(in_names)
    all_names = in_names + out_names
    if pname is not None:
        all_names = all_names + [pname]

    def _body(*args):
        operands = list(args)
        if pname is not None:
            operands.append(bass2jax.partition_id_tensor())
        outs = bass2jax._bass_exec_p.bind(
            *operands,
            out_avals=tuple(out_avals),
            in_names=tuple(all_names),
            out_names=tuple(out_names),
            lowering_input_output_aliases=(),
            sim_require_finite=True,
            sim_require_nnan=True,
            nc=nc,
        )
        return tuple(outs)

    devices = jax.devices()[:n_cores]
    mesh = Mesh(np.asarray(devices), ("core",))
    n_out = len(out_names)
    sharded_jit = jax.jit(
        shard_map(_body, mesh=mesh,
                  in_specs=(PartitionSpec("core"),) * (n_params + n_out),
                  out_specs=(PartitionSpec("core"),) * n_out,
                  check_rep=False),
        keep_unused=True,
    )

    zero_outs = [
        jax.device_put(
            np.zeros((n_cores * a.shape[0], *a.shape[1:]), a.dtype),
            NamedSharding(mesh, PartitionSpec("core")),
        )
        for a in out_avals
    ]

    compiled = {"fn": None}

    def run(in_maps):
        concat_in = [
            np.concatenate([np.asarray(m[name]) for m in in_maps], axis=0)
            for name in in_names
        ]
        args = [jax.device_put(a) for a in concat_in]
        if compiled["fn"] is None:
            compiled["fn"] = bass2jax.fast_dispatch_compile(
                lambda: sharded_jit.lower(*args, *zero_outs).compile())
        out_arrs = compiled["fn"](*args, *zero_outs)
        import jax as _jax
        _jax.block_until_ready(out_arrs)
        return [
            {name: np.asarray(out_arrs[i]).reshape(n_cores, *out_avals[i].shape)[c]
             for i, name in enumerate(out_names)}
            for c in range(n_cores)
        ]

    run.in_names = in_names
    run.out_names = out_names
    run.out_avals = out_avals
    run.zero_outs = zero_outs
    run.compiled = compiled
    run.n_cores = n_cores
    _RUNNER_CACHE[gpc] = run
    return run


GPC = 8  # 1 core does the whole problem: per-exec dispatch overhead on the
         # axon path scales with cores x operands and dwarfs device exec
         # (~1.2 ms); measured fastest of the 1/2/8-core variants.


def kernel(x, Wq, bq, Wk, bk, Wv, bv, Wo, bo, **_ignored):
    x = np.asarray(x, dtype=np.float32)
    in_maps = make_in_maps(
        x,
        np.asarray(Wq, np.float32), np.asarray(bq, np.float32),
        np.asarray(Wk, np.float32), np.asarray(bk, np.float32),
        np.asarray(Wv, np.float32), np.asarray(bv, np.float32),
        np.asarray(Wo, np.float32), gpc=GPC,
    )
    try:
        results = get_runner(GPC)(in_maps)
    except Exception:
        # fallback: stock SPMD runner (slower dispatch, same NEFF)
        from concourse.bass_utils import run_bass_kernel_spmd
        results = run_bass_kernel_spmd(
            get_nc(GPC), in_maps, core_ids=list(range(8 // GPC))).results
    return combine_results(results, np.asarray(bo, np.float32), gpc=GPC)


# revision 4
# speedup vs baseline: 39.0765x; 1.0165x over previous
"""Causal self-attention (D=1024, H=16, S=2048, B=2) — dispatch-lean rewrite.

Design notes (vs the staged baseline kernel.py):
- The dominant cost on this axon-tunneled setup is per-exec dispatch work
  proportional to (#operands x #cores), not device compute. So ALL inputs
  are packed into ONE bf16 blob per core (pre-transposed x, weight slices,
  and biases, in exactly the SBUF layouts the kernel wants) — a single
  input operand, no DMA transposes on device.
- GPC = head-groups per core; n_cores = 8 // GPC. Each core processes
  GPC (batch, head-group) pairs sequentially; 4 head-groups of 4 heads each
  per batch. GPC=8 (one core) measures fastest: per-exec dispatch overhead
  scales with cores and dwarfs the ~1.2 ms single-core device exec.
- Softmax denominator broadcast uses gpsimd partition_broadcast (SBUF->SBUF)
  instead of the baseline's DRAM bounce.
- Runner compiled via bass2jax.fast_dispatch_compile (C++ fast-path dispatch);
  enable_partition_id=False drops the partition-id operand.

Blob layout per core ([128, COLS] bf16):
  for each local batch lb: xT region [128, 8, 2048]  (xT[p,c,s] = x[b,s,128c+p])
  then per local group gi:
    wqkv [128, 3, 8, 256]  (w[p,j,o,d] = W_j[128o+p, 256g+d], j in q,k,v)
    wo   [128, 2, 1024]    (wo[p,dc,e] = Wo[256g+128dc+p, e])
    bias [128, 512]        ([bq 2][bk 2][bv 256, row-replicated][pad])
Output y [n_b*2048, 1024] f32; host adds bo (and sums partials when GPC<4).
"""

import sys

sys.path.insert(0, "/opt/trn_rl_repo")

import ml_dtypes
import numpy as np

import concourse.bass as bass
import concourse.mybir as mybir
import concourse.tile as tile
from concourse import bacc

P = 128
S = 2048
D = 1024
NH = 4                    # heads per group
DH = 64                   # head dim
DPC = NH * DH             # model dims per group = 256
N_CT = D // P             # 8 contraction chunks
N_ST = S // P             # 16 t tiles of 128
N_SB = S // 512           # 4 s blocks of 512
F32 = mybir.dt.float32
BF16 = mybir.dt.bfloat16
SCALE = 1.0 / 8.0         # 1/sqrt(64)
XCOLS = N_CT * S          # 16384 cols per batch xT region
BIAS_COLS = 512           # [bq 2][bk 2][bv 256][pad] bf16, bv replicated per row
WCOLS = 3 * N_CT * DPC + 2 * D + BIAS_COLS   # 8704 cols per group

AF = mybir.ActivationFunctionType
ALU = mybir.AluOpType

ALL_PAIRS = [(b, g) for b in range(2) for g in range(4)]


def core_pairs(gpc: int, core: int):
    return ALL_PAIRS[core * gpc:(core + 1) * gpc]


def build_nc(gpc: int) -> bass.Bass:
    """One NEFF shared by all cores of the gpc config (SPMD, identical
    structure; only the data differs)."""
    assert gpc in (1, 2, 4, 8)
    pairs = core_pairs(gpc, 0)
    local_batches = sorted({b for b, _ in pairs})
    n_b = len(local_batches)

    nc = bacc.Bacc("TRN2", target_bir_lowering=False, debug=False,
                   num_devices=8 // gpc, enable_partition_id=False)

    blob_d = nc.dram_tensor("blob", [P, n_b * XCOLS + gpc * WCOLS], BF16,
                            kind="ExternalInput")
    y_d = nc.dram_tensor("y", [n_b * S, D], F32, kind="ExternalOutput")
    wbase0 = n_b * XCOLS

    with tile.TileContext(nc) as tc:
        with (
            tc.tile_pool(name="const", bufs=2) as const,
            tc.tile_pool(name="xtp", bufs=1) as xtp,
            tc.tile_pool(name="wp", bufs=2) as wp,
            tc.tile_pool(name="wop", bufs=min(gpc, 4) + 1) as wop,
            tc.tile_pool(name="qkv", bufs=2) as qkv,
            tc.tile_pool(name="atp", bufs=min(gpc, 4)) as atp,
            tc.tile_pool(name="work", bufs=3) as work,
            tc.tile_pool(name="att", bufs=4) as attw,
            tc.tile_pool(name="denp", bufs=4) as denp,
            tc.tile_pool(name="rbp", bufs=4) as rbp,
            tc.tile_pool(name="ps", bufs=2, space="PSUM") as psp,
            tc.tile_pool(name="ppv", bufs=4, space="PSUM") as ppv,
        ):
            for lb, b in enumerate(local_batches):
                # ---- xT for this batch: one 4MB DMA ----
                xT = xtp.tile([P, N_CT, S], BF16, tag="xt", name=f"xt{lb}")
                nc.sync.dma_start(
                    xT, blob_d[:, lb * XCOLS:(lb + 1) * XCOLS]
                    .rearrange("p (c s) -> p c s", c=N_CT))

                b_groups = [i for i, (bb, _) in enumerate(pairs) if bb == b]
                ATs = {}
                wos = {}
                for gi in b_groups:
                    wb = wbase0 + gi * WCOLS
                    w = wp.tile([P, 3, N_CT, DPC], BF16, tag="w", name=f"w{gi}")
                    nc.sync.dma_start(
                        w, blob_d[:, wb:wb + 3 * N_CT * DPC]
                        .rearrange("p (j o d) -> p j o d", j=3, o=N_CT))
                    wo_s = wop.tile([P, 2, D], BF16, tag="wo", name=f"wo{gi}")
                    wo0 = wb + 3 * N_CT * DPC
                    nc.sync.dma_start(
                        wo_s, blob_d[:, wo0:wo0 + 2 * D]
                        .rearrange("p (dc e) -> p dc e", dc=2))
                    wos[gi] = wo_s

                    # biases for this group: bf16 in blob -> f32 on DVE
                    bb = const.tile([P, 260], BF16, tag="bb")
                    nc.sync.dma_start(bb, blob_d[:, wo0 + 2 * D:
                                                 wo0 + 2 * D + 260])
                    bf_t = const.tile([P, 260], F32, tag="bf")
                    nc.vector.tensor_copy(bf_t, bb)
                    bqk_sc = bf_t[:, 0:4].rearrange("p (j o) -> p j o", j=2)
                    bv_v = bf_t[:, 4:260]

                    # ---- phase 1: projections ----
                    QT = qkv.tile([P, 2, S], BF16, tag="qt", name=f"qt{gi}")
                    KT = qkv.tile([P, 2, S], BF16, tag="kt", name=f"kt{gi}")
                    vaug = qkv.tile([P, N_ST, NH, DH + 1], BF16,
                                    tag="va", name=f"va{gi}")
                    nc.vector.memset(vaug[:, :, :, DH:DH + 1], 1.0)
                    vaug_v = vaug[:, :, :, 0:DH]

                    for sb in range(N_SB):
                        for dc in range(2):
                            for j, dst in ((0, QT), (1, KT)):
                                ps = psp.tile([P, 2, 512], F32, tag="ps")
                                for c in range(N_CT):
                                    nc.tensor.matmul(
                                        ps[:, 0, :],
                                        w[:, j, c, dc * P:(dc + 1) * P],
                                        xT[:, c, sb * 512:(sb + 1) * 512],
                                        start=(c == 0),
                                        stop=(c == N_CT - 1),
                                    )
                                nc.vector.tensor_scalar_add(
                                    dst[:, dc, sb * 512:(sb + 1) * 512],
                                    ps[:, 0, :],
                                    bqk_sc[:, j, dc:dc + 1],
                                )

                    for tt in range(N_ST):
                        ps = psp.tile([P, 2, 512], F32, tag="ps")
                        pvs = ps[:, 0, 0:DPC]
                        for c in range(N_CT):
                            nc.tensor.matmul(
                                pvs,
                                xT[:, c, tt * P:(tt + 1) * P],
                                w[:, 2, c, :],
                                start=(c == 0),
                                stop=(c == N_CT - 1),
                            )
                        nc.vector.tensor_add(
                            vaug_v[:, tt, :, :],
                            pvs.rearrange("p (h u) -> p h u", h=NH),
                            bv_v.rearrange("p (h u) -> p h u", h=NH),
                        )

                    # ---- phase 2: attention ----
                    AT = atp.tile([P, 2, S], BF16, tag="at", name=f"at{gi}")
                    ATs[gi] = AT
                    for dc in range(2):
                        for sb in range(N_SB):
                            pvs2 = [ppv.tile([DH + 1, 512], F32, tag="pv",
                                             name=f"pv{gi}_{dc}_{sb}_{e}")
                                    for e in range(2)]
                            t_cnt = 4 * sb + 4
                            for T in range(t_cnt):
                                k = T - 4 * sb
                                ms = 128 * k if k > 0 else 0
                                sc = psp.tile([P, 2, 512], F32, tag="ps")
                                ex = attw.tile([P, 2, 512], BF16, tag="ex")
                                for e in range(2):
                                    off = DH * e
                                    nc.tensor.matmul(
                                        sc[:, e, ms:512],
                                        KT[off:off + DH, dc, T * P:(T + 1) * P],
                                        QT[off:off + DH, dc,
                                           sb * 512 + ms:(sb + 1) * 512],
                                        start=True,
                                        stop=True,
                                    )
                                nc.scalar.activation(
                                    ex[:, :, ms:512], sc[:, :, ms:512],
                                    AF.Exp, scale=SCALE,
                                )
                                if k >= 0:
                                    nc.gpsimd.affine_select(
                                        out=ex[:, :, ms:ms + P],
                                        in_=ex[:, :, ms:ms + P],
                                        compare_op=ALU.is_ge,
                                        fill=0.0,
                                        base=0,
                                        pattern=[[0, 2], [1, P]],
                                        channel_multiplier=-1,
                                    )
                                for e in range(2):
                                    h = 2 * dc + e
                                    nc.tensor.matmul(
                                        pvs2[e][:, ms:512],
                                        vaug[:, T, h, :],
                                        ex[:, e, ms:512],
                                        start=(T == 0),
                                        stop=(T == t_cnt - 1),
                                    )
                            for e in range(2):
                                pv = pvs2[e]
                                # row 64 of pv is the softmax denominator
                                den = denp.tile([1, 512], F32, name="den")
                                nc.vector.reciprocal(
                                    out=den, in_=pv[DH:DH + 1, :])
                                rb = rbp.tile([DH, 512], F32)
                                nc.gpsimd.partition_broadcast(rb, den[0:1, :])
                                if e == 0:
                                    nc.vector.tensor_mul(
                                        AT[0:DH, dc, sb * 512:(sb + 1) * 512],
                                        pv[0:DH, :], rb)
                                else:
                                    att = attw.tile([DH, 512], BF16, tag="att")
                                    nc.vector.tensor_mul(att, pv[0:DH, :], rb)
                                    nc.sync.dma_start(
                                        AT[DH:P, dc, sb * 512:(sb + 1) * 512],
                                        att)

                # ---- phase 3: output projection for this batch ----
                kparts = [(gi, dc) for gi in b_groups for dc in range(2)]
                for st in range(N_ST):
                    ps = psp.tile([P, 2, 512], F32, tag="ps")
                    for eb in range(2):
                        for ki, (gi2, dc) in enumerate(kparts):
                            nc.tensor.matmul(
                                ps[:, eb, :],
                                ATs[gi2][:, dc, st * P:(st + 1) * P],
                                wos[gi2][:, dc, eb * 512:(eb + 1) * 512],
                                start=(ki == 0),
                                stop=(ki == len(kparts) - 1),
                            )
                    ys = work.tile([P, D], F32, tag="work")
                    nc.scalar.copy(ys[:, 0:512], ps[:, 0, :])
                    nc.scalar.copy(ys[:, 512:1024], ps[:, 1, :])
                    nc.sync.dma_start(
                        y_d[lb * S + st * P:lb * S + (st + 1) * P, :], ys)

    nc.finalize()
    return nc


_NC_CACHE = {}


def get_nc(gpc: int):
    if gpc not in _NC_CACHE:
        _NC_CACHE[gpc] = build_nc(gpc)
    return _NC_CACHE[gpc]


def make_in_maps(x, Wq, bq, Wk, bk, Wv, bv, Wo, gpc: int):
    """Per-core packed input dicts for the gpc config."""
    bf = ml_dtypes.bfloat16
    n_cores = 8 // gpc
    xT = {}  # b -> [128, 8*2048] bf16
    for b in range(2):
        t = np.ascontiguousarray(x[b].T)          # [1024, 2048]
        t = t.reshape(N_CT, P, S).transpose(1, 0, 2).reshape(P, XCOLS)
        xT[b] = t.astype(bf)

    def wslice(W, g):  # [128, 8, 256] -> [128, 8*256]
        return (W[:, g * DPC:(g + 1) * DPC]
                .reshape(N_CT, P, DPC).transpose(1, 0, 2).reshape(P, -1))

    def woslice(W, g):  # [128, 2, 1024] -> [128, 2*1024]
        return (W[g * DPC:(g + 1) * DPC, :]
                .reshape(2, P, D).transpose(1, 0, 2).reshape(P, -1))

    def biasblock(g):  # [128, 512]: [bq 2][bk 2][bv 256][pad]
        blk = np.zeros((P, BIAS_COLS), np.float32)
        blk[:, 0:2] = bq[g * DPC:(g + 1) * DPC].reshape(2, P).T
        blk[:, 2:4] = bk[g * DPC:(g + 1) * DPC].reshape(2, P).T
        blk[:, 4:260] = bv[g * DPC:(g + 1) * DPC][None, :]
        return blk

    in_maps = []
    for core in range(n_cores):
        pairs = core_pairs(gpc, core)
        local_batches = sorted({b for b, _ in pairs})
        pieces = [xT[b] for b in local_batches]
        for (b, g) in pairs:
            pieces += [
                wslice(Wq, g).astype(bf), wslice(Wk, g).astype(bf),
                wslice(Wv, g).astype(bf), woslice(Wo, g).astype(bf),
                biasblock(g).astype(bf),
            ]
        blob = np.ascontiguousarray(np.concatenate(pieces, axis=1))
        in_maps.append({"blob": blob})
    return in_maps


def combine_results(results, bo, gpc: int):
    out = np.zeros((2, S, D), dtype=np.float32)
    n_cores = 8 // gpc
    for core in range(n_cores):
        pairs = core_pairs(gpc, core)
        local_batches = sorted({b for b, _ in pairs})
        y = results[core]["y"].reshape(len(local_batches), S, D)
        for lb, b in enumerate(local_batches):
            out[b] += y[lb]
    out += bo.astype(np.float32)
    return out


_RUNNER_CACHE = {}


def get_runner(gpc: int):
    """Fast-dispatch jitted runner over 8//gpc cores."""
    if gpc in _RUNNER_CACHE:
        return _RUNNER_CACHE[gpc]

    import jax
    from jax.sharding import Mesh, PartitionSpec, NamedSharding
    from jax.experimental.shard_map import shard_map
    from concourse import bass2jax, mybir as _mb

    nc = get_nc(gpc)
    bass2jax.install_neuronx_cc_hook()
    n_cores = 8 // gpc

    pname = nc.partition_id_tensor.name if nc.partition_id_tensor else None
    in_names, out_names, out_avals = [], [], []
    for alloc in nc.m.functions[0].allocations:
        if not isinstance(alloc, _mb.MemoryLocationSet):
            continue
        name = alloc.memorylocations[0].name
        if alloc.kind == "ExternalInput":
            if name != pname:
                in_names.append(name)
        elif alloc.kind == "ExternalOutput":
            out_names.append(name)
            out_avals.append(jax.core.ShapedArray(
                tuple(alloc.tensor_shape), _mb.dt.np(alloc.dtype)))
    n_params = len(in_names)
    all_names = in_names + out_names
    if pname is not None:
        all_names = all_names + [pname]

    def _body(*args):
        operands = list(args)
        if pname is not None:
            operands.append(bass2jax.partition_id_tensor())
        outs = bass2jax._bass_exec_p.bind(
            *operands,
            out_avals=tuple(out_avals),
            in_names=tuple(all_names),
            out_names=tuple(out_names),
            lowering_input_output_aliases=(),
            sim_require_finite=True,
            sim_require_nnan=True,
            nc=nc,
        )
        return tuple(outs)

    devices = jax.devices()[:n_cores]
    mesh = Mesh(np.asarray(devices), ("core",))
    n_out = len(out_names)
    sharded_jit = jax.jit(
        shard_map(_body, mesh=mesh,
                  in_specs=(PartitionSpec("core"),) * (n_params + n_out),
                  out_specs=(PartitionSpec("core"),) * n_out,
                  check_rep=False),
        keep_unused=True,
    )

    zero_outs = [
        jax.device_put(
            np.zeros((n_cores * a.shape[0], *a.shape[1:]), a.dtype),
            NamedSharding(mesh, PartitionSpec("core")),
        )
        for a in out_avals
    ]

    compiled = {"fn": None}

    def run(in_maps):
        concat_in = [
            np.concatenate([np.asarray(m[name]) for m in in_maps], axis=0)
            for name in in_names
        ]
        args = [jax.device_put(a) for a in concat_in]
        if compiled["fn"] is None:
            compiled["fn"] = bass2jax.fast_dispatch_compile(
                lambda: sharded_jit.lower(*args, *zero_outs).compile())
        out_arrs = compiled["fn"](*args, *zero_outs)
        import jax as _jax
        _jax.block_until_ready(out_arrs)
        return [
            {name: np.asarray(out_arrs[i]).reshape(n_cores, *out_avals[i].shape)[c]
             for i, name in enumerate(out_names)}
            for c in range(n_cores)
        ]

    run.in_names = in_names
    run.out_names = out_names
    run.out_avals = out_avals
    run.zero_outs = zero_outs
    run.compiled = compiled
    run.n_cores = n_cores
    _RUNNER_CACHE[gpc] = run
    return run


GPC = 8  # 1 core does the whole problem: per-exec dispatch overhead on the
         # axon path scales with cores x operands and dwarfs device exec
         # (~1.2 ms); measured fastest of the 1/2/8-core variants.


def kernel(x, Wq, bq, Wk, bk, Wv, bv, Wo, bo, **_ignored):
    x = np.asarray(x, dtype=np.float32)
    in_maps = make_in_maps(
        x,
        np.asarray(Wq, np.float32), np.asarray(bq, np.float32),
        np.asarray(Wk, np.float32), np.asarray(bk, np.float32),
        np.asarray(Wv, np.float32), np.asarray(bv, np.float32),
        np.asarray(Wo, np.float32), gpc=GPC,
    )
    try:
        results = get_runner(GPC)(in_maps)
    except Exception:
        # fallback: stock SPMD runner (slower dispatch, same NEFF)
        from concourse.bass_utils import run_bass_kernel_spmd
        results = run_bass_kernel_spmd(
            get_nc(GPC), in_maps, core_ids=list(range(8 // GPC))).results
    return combine_results(results, np.asarray(bo, np.float32), gpc=GPC)


# revision 5
# speedup vs baseline: 42.4927x; 1.0874x over previous
"""Causal self-attention (D=1024, H=16, S=2048, B=2) — dispatch-lean rewrite.

Design notes (vs the staged baseline kernel.py):
- The dominant cost on this axon-tunneled setup is per-exec dispatch work
  proportional to (#operands x #cores), not device compute. So ALL inputs
  are packed into ONE bf16 blob per core (pre-transposed x, weight slices,
  and biases, in exactly the SBUF layouts the kernel wants) — a single
  input operand, no DMA transposes on device.
- GPC = head-groups per core; n_cores = 8 // GPC. Each core processes
  GPC (batch, head-group) pairs sequentially; 4 head-groups of 4 heads each
  per batch. GPC=8 (one core) measures fastest: per-exec dispatch overhead
  scales with cores and dwarfs the ~1.2 ms single-core device exec.
- Softmax denominator broadcast uses gpsimd partition_broadcast (SBUF->SBUF)
  instead of the baseline's DRAM bounce.
- Runner compiled via bass2jax.fast_dispatch_compile (C++ fast-path dispatch);
  enable_partition_id=False drops the partition-id operand.

Blob layout per core ([128, COLS] bf16):
  for each local batch lb: xT region [128, 8, 2048]  (xT[p,c,s] = x[b,s,128c+p])
  then per local group gi:
    wqkv [128, 3, 8, 256]  (w[p,j,o,d] = W_j[128o+p, 256g+d], j in q,k,v)
    wo   [128, 2, 1024]    (wo[p,dc,e] = Wo[256g+128dc+p, e])
    bias [128, 512]        ([bq 2][bk 2][bv 256, row-replicated][pad])
Output y [n_b*2048, 1024] bf16 (~0.2% err, gate is 2e-2); host upcasts,
adds bo (and sums partials when GPC<4).
"""

import sys

sys.path.insert(0, "/opt/trn_rl_repo")

import ml_dtypes
import numpy as np

import concourse.bass as bass
import concourse.mybir as mybir
import concourse.tile as tile
from concourse import bacc

P = 128
S = 2048
D = 1024
NH = 4                    # heads per group
DH = 64                   # head dim
DPC = NH * DH             # model dims per group = 256
N_CT = D // P             # 8 contraction chunks
N_ST = S // P             # 16 t tiles of 128
N_SB = S // 512           # 4 s blocks of 512
F32 = mybir.dt.float32
BF16 = mybir.dt.bfloat16
SCALE = 1.0 / 8.0         # 1/sqrt(64)
XCOLS = N_CT * S          # 16384 cols per batch xT region
BIAS_COLS = 512           # [bq 2][bk 2][bv 256][pad] bf16, bv replicated per row
WCOLS = 3 * N_CT * DPC + 2 * D + BIAS_COLS   # 8704 cols per group

AF = mybir.ActivationFunctionType
ALU = mybir.AluOpType

ALL_PAIRS = [(b, g) for b in range(2) for g in range(4)]


def core_pairs(gpc: int, core: int):
    return ALL_PAIRS[core * gpc:(core + 1) * gpc]


def build_nc(gpc: int) -> bass.Bass:
    """One NEFF shared by all cores of the gpc config (SPMD, identical
    structure; only the data differs)."""
    assert gpc in (1, 2, 4, 8)
    pairs = core_pairs(gpc, 0)
    local_batches = sorted({b for b, _ in pairs})
    n_b = len(local_batches)

    nc = bacc.Bacc("TRN2", target_bir_lowering=False, debug=False,
                   num_devices=8 // gpc, enable_partition_id=False)

    blob_d = nc.dram_tensor("blob", [P, n_b * XCOLS + gpc * WCOLS], BF16,
                            kind="ExternalInput")
    y_d = nc.dram_tensor("y", [n_b * S, D], BF16, kind="ExternalOutput")
    wbase0 = n_b * XCOLS

    with tile.TileContext(nc) as tc:
        with (
            tc.tile_pool(name="const", bufs=2) as const,
            tc.tile_pool(name="xtp", bufs=1) as xtp,
            tc.tile_pool(name="wp", bufs=2) as wp,
            tc.tile_pool(name="wop", bufs=min(gpc, 4) + 1) as wop,
            tc.tile_pool(name="qkv", bufs=2) as qkv,
            tc.tile_pool(name="atp", bufs=min(gpc, 4)) as atp,
            tc.tile_pool(name="work", bufs=2) as work,
            tc.tile_pool(name="att", bufs=4) as attw,
            tc.tile_pool(name="denp", bufs=4) as denp,
            tc.tile_pool(name="rbp", bufs=4) as rbp,
            tc.tile_pool(name="ps", bufs=2, space="PSUM") as psp,
            tc.tile_pool(name="ppv", bufs=4, space="PSUM") as ppv,
        ):
            for lb, b in enumerate(local_batches):
                # ---- xT for this batch: one 4MB DMA ----
                xT = xtp.tile([P, N_CT, S], BF16, tag="xt", name=f"xt{lb}")
                nc.sync.dma_start(
                    xT, blob_d[:, lb * XCOLS:(lb + 1) * XCOLS]
                    .rearrange("p (c s) -> p c s", c=N_CT))

                b_groups = [i for i, (bb, _) in enumerate(pairs) if bb == b]
                ATs = {}
                wos = {}
                for gi in b_groups:
                    wb = wbase0 + gi * WCOLS
                    # one DMA: wqkv (6144) + bias block (first 260 of 512)
                    wqb = wp.tile([P, 3 * N_CT * DPC + 260], BF16, tag="w",
                                  name=f"w{gi}")
                    nc.sync.dma_start(
                        wqb, blob_d[:, wb:wb + 3 * N_CT * DPC + 260])
                    w = wqb[:, 0:3 * N_CT * DPC].rearrange(
                        "p (j o d) -> p j o d", j=3, o=N_CT)
                    wo_s = wop.tile([P, 2, D], BF16, tag="wo", name=f"wo{gi}")
                    wo0 = wb + 3 * N_CT * DPC + BIAS_COLS
                    nc.sync.dma_start(
                        wo_s, blob_d[:, wo0:wo0 + 2 * D]
                        .rearrange("p (dc e) -> p dc e", dc=2))
                    wos[gi] = wo_s

                    # biases: bf16 tail of the wqb tile -> f32 on DVE
                    bf_t = const.tile([P, 260], F32, tag="bf")
                    nc.vector.tensor_copy(bf_t, wqb[:, 3 * N_CT * DPC:])
                    bqk_sc = bf_t[:, 0:4].rearrange("p (j o) -> p j o", j=2)
                    bv_v = bf_t[:, 4:260]

                    # ---- phase 1: projections ----
                    QT = qkv.tile([P, 2, S], BF16, tag="qt", name=f"qt{gi}")
                    KT = qkv.tile([P, 2, S], BF16, tag="kt", name=f"kt{gi}")
                    vaug = qkv.tile([P, N_ST, NH, DH + 1], BF16,
                                    tag="va", name=f"va{gi}")
                    nc.vector.memset(vaug[:, :, :, DH:DH + 1], 1.0)
                    vaug_v = vaug[:, :, :, 0:DH]

                    for sb in range(N_SB):
                        for dc in range(2):
                            for j, dst in ((0, QT), (1, KT)):
                                ps = psp.tile([P, 2, 512], F32, tag="ps")
                                for c in range(N_CT):
                                    nc.tensor.matmul(
                                        ps[:, 0, :],
                                        w[:, j, c, dc * P:(dc + 1) * P],
                                        xT[:, c, sb * 512:(sb + 1) * 512],
                                        start=(c == 0),
                                        stop=(c == N_CT - 1),
                                    )
                                nc.vector.tensor_scalar_add(
                                    dst[:, dc, sb * 512:(sb + 1) * 512],
                                    ps[:, 0, :],
                                    bqk_sc[:, j, dc:dc + 1],
                                )

                    for tt in range(N_ST):
                        ps = psp.tile([P, 2, 512], F32, tag="ps")
                        pvs = ps[:, 0, 0:DPC]
                        for c in range(N_CT):
                            nc.tensor.matmul(
                                pvs,
                                xT[:, c, tt * P:(tt + 1) * P],
                                w[:, 2, c, :],
                                start=(c == 0),
                                stop=(c == N_CT - 1),
                            )
                        nc.vector.tensor_add(
                            vaug_v[:, tt, :, :],
                            pvs.rearrange("p (h u) -> p h u", h=NH),
                            bv_v.rearrange("p (h u) -> p h u", h=NH),
                        )

                    # ---- phase 2: attention ----
                    AT = atp.tile([P, 2, S], BF16, tag="at", name=f"at{gi}")
                    ATs[gi] = AT
                    for dc in range(2):
                        for sb in range(N_SB):
                            pvs2 = [ppv.tile([DH + 1, 512], F32, tag="pv",
                                             name=f"pv{gi}_{dc}_{sb}_{e}")
                                    for e in range(2)]
                            t_cnt = 4 * sb + 4
                            # software-pipelined: PV(T-1) is emitted after
                            # score/exp(T) so PE is not head-of-line blocked
                            # on the exp of the same iteration.
                            pend = None  # (ex, ms, T) awaiting PV
                            for T in range(t_cnt):
                                k = T - 4 * sb
                                ms = 128 * k if k > 0 else 0
                                sc = psp.tile([P, 2, 512], F32, tag="ps")
                                ex = attw.tile([P, 2, 512], BF16, tag="ex")
                                for e in range(2):
                                    off = DH * e
                                    nc.tensor.matmul(
                                        sc[:, e, ms:512],
                                        KT[off:off + DH, dc, T * P:(T + 1) * P],
                                        QT[off:off + DH, dc,
                                           sb * 512 + ms:(sb + 1) * 512],
                                        start=True,
                                        stop=True,
                                    )
                                nc.scalar.activation(
                                    ex[:, :, ms:512], sc[:, :, ms:512],
                                    AF.Exp, scale=SCALE,
                                )
                                if k >= 0:
                                    nc.gpsimd.affine_select(
                                        out=ex[:, :, ms:ms + P],
                                        in_=ex[:, :, ms:ms + P],
                                        compare_op=ALU.is_ge,
                                        fill=0.0,
                                        base=0,
                                        pattern=[[0, 2], [1, P]],
                                        channel_multiplier=-1,
                                    )
                                if pend is not None:
                                    pex, pms, pT = pend
                                    for e in range(2):
                                        h = 2 * dc + e
                                        nc.tensor.matmul(
                                            pvs2[e][:, pms:512],
                                            vaug[:, pT, h, :],
                                            pex[:, e, pms:512],
                                            start=(pT == 0),
                                            stop=False,
                                        )
                                pend = (ex, ms, T)
                            pex, pms, pT = pend
                            for e in range(2):
                                h = 2 * dc + e
                                nc.tensor.matmul(
                                    pvs2[e][:, pms:512],
                                    vaug[:, pT, h, :],
                                    pex[:, e, pms:512],
                                    start=(pT == 0),
                                    stop=True,
                                )
                            for e in range(2):
                                pv = pvs2[e]
                                # row 64 of pv is the softmax denominator
                                den = denp.tile([1, 512], F32, name="den")
                                nc.vector.reciprocal(
                                    out=den, in_=pv[DH:DH + 1, :])
                                rb = rbp.tile([DH, 512], F32)
                                nc.gpsimd.partition_broadcast(rb, den[0:1, :])
                                if e == 0:
                                    nc.vector.tensor_mul(
                                        AT[0:DH, dc, sb * 512:(sb + 1) * 512],
                                        pv[0:DH, :], rb)
                                else:
                                    att = attw.tile([DH, 512], BF16, tag="att")
                                    nc.vector.tensor_mul(att, pv[0:DH, :], rb)
                                    nc.sync.dma_start(
                                        AT[DH:P, dc, sb * 512:(sb + 1) * 512],
                                        att)

                # ---- phase 3: output projection for this batch ----
                kparts = [(gi, dc) for gi in b_groups for dc in range(2)]
                for st2 in range(N_ST // 2):
                    ys = work.tile([P, 2, D], BF16, tag="work")
                    for half in range(2):
                        st = 2 * st2 + half
                        ps = psp.tile([P, 2, 512], F32, tag="ps")
                        for eb in range(2):
                            for ki, (gi2, dc) in enumerate(kparts):
                                nc.tensor.matmul(
                                    ps[:, eb, :],
                                    ATs[gi2][:, dc, st * P:(st + 1) * P],
                                    wos[gi2][:, dc, eb * 512:(eb + 1) * 512],
                                    start=(ki == 0),
                                    stop=(ki == len(kparts) - 1),
                                )
                        nc.scalar.copy(ys[:, half, 0:512], ps[:, 0, :])
                        nc.scalar.copy(ys[:, half, 512:1024], ps[:, 1, :])
                    r0 = lb * S + st2 * 2 * P
                    nc.sync.dma_start(
                        y_d[r0:r0 + 2 * P, :]
                        .rearrange("(t p) e -> p t e", p=P), ys)

    nc.finalize()
    return nc


_NC_CACHE = {}


def get_nc(gpc: int):
    if gpc not in _NC_CACHE:
        _NC_CACHE[gpc] = build_nc(gpc)
    return _NC_CACHE[gpc]


def make_in_maps(x, Wq, bq, Wk, bk, Wv, bv, Wo, gpc: int):
    """Per-core packed input dicts for the gpc config."""
    bf = ml_dtypes.bfloat16
    n_cores = 8 // gpc
    xT = {}  # b -> [128, 8*2048] bf16
    for b in range(2):
        t = np.ascontiguousarray(x[b].T)          # [1024, 2048]
        t = t.reshape(N_CT, P, S).transpose(1, 0, 2).reshape(P, XCOLS)
        xT[b] = t.astype(bf)

    def wslice(W, g):  # [128, 8, 256] -> [128, 8*256]
        return (W[:, g * DPC:(g + 1) * DPC]
                .reshape(N_CT, P, DPC).transpose(1, 0, 2).reshape(P, -1))

    def woslice(W, g):  # [128, 2, 1024] -> [128, 2*1024]
        return (W[g * DPC:(g + 1) * DPC, :]
                .reshape(2, P, D).transpose(1, 0, 2).reshape(P, -1))

    def biasblock(g):  # [128, 512]: [bq 2][bk 2][bv 256][pad]
        blk = np.zeros((P, BIAS_COLS), np.float32)
        blk[:, 0:2] = bq[g * DPC:(g + 1) * DPC].reshape(2, P).T
        blk[:, 2:4] = bk[g * DPC:(g + 1) * DPC].reshape(2, P).T
        blk[:, 4:260] = bv[g * DPC:(g + 1) * DPC][None, :]
        return blk

    in_maps = []
    for core in range(n_cores):
        pairs = core_pairs(gpc, core)
        local_batches = sorted({b for b, _ in pairs})
        pieces = [xT[b] for b in local_batches]
        for (b, g) in pairs:
            pieces += [
                wslice(Wq, g).astype(bf), wslice(Wk, g).astype(bf),
                wslice(Wv, g).astype(bf), biasblock(g).astype(bf),
                woslice(Wo, g).astype(bf),
            ]
        blob = np.ascontiguousarray(np.concatenate(pieces, axis=1))
        in_maps.append({"blob": blob})
    return in_maps


def combine_results(results, bo, gpc: int):
    out = np.zeros((2, S, D), dtype=np.float32)
    n_cores = 8 // gpc
    for core in range(n_cores):
        pairs = core_pairs(gpc, core)
        local_batches = sorted({b for b, _ in pairs})
        y = results[core]["y"].reshape(len(local_batches), S, D)
        for lb, b in enumerate(local_batches):
            out[b] += y[lb]
    out += bo.astype(np.float32)
    return out


_RUNNER_CACHE = {}


def get_runner(gpc: int):
    """Fast-dispatch jitted runner over 8//gpc cores."""
    if gpc in _RUNNER_CACHE:
        return _RUNNER_CACHE[gpc]

    import jax
    from jax.sharding import Mesh, PartitionSpec, NamedSharding
    from jax.experimental.shard_map import shard_map
    from concourse import bass2jax, mybir as _mb

    nc = get_nc(gpc)
    bass2jax.install_neuronx_cc_hook()
    n_cores = 8 // gpc

    pname = nc.partition_id_tensor.name if nc.partition_id_tensor else None
    in_names, out_names, out_avals = [], [], []
    for alloc in nc.m.functions[0].allocations:
        if not isinstance(alloc, _mb.MemoryLocationSet):
            continue
        name = alloc.memorylocations[0].name
        if alloc.kind == "ExternalInput":
            if name != pname:
                in_names.append(name)
        elif alloc.kind == "ExternalOutput":
            out_names.append(name)
            out_avals.append(jax.core.ShapedArray(
                tuple(alloc.tensor_shape), _mb.dt.np(alloc.dtype)))
    n_params = len(in_names)
    all_names = in_names + out_names
    if pname is not None:
        all_names = all_names + [pname]

    def _body(*args):
        operands = list(args)
        if pname is not None:
            operands.append(bass2jax.partition_id_tensor())
        outs = bass2jax._bass_exec_p.bind(
            *operands,
            out_avals=tuple(out_avals),
            in_names=tuple(all_names),
            out_names=tuple(out_names),
            lowering_input_output_aliases=(),
            sim_require_finite=True,
            sim_require_nnan=True,
            nc=nc,
        )
        return tuple(outs)

    devices = jax.devices()[:n_cores]
    mesh = Mesh(np.asarray(devices), ("core",))
    n_out = len(out_names)
    sharded_jit = jax.jit(
        shard_map(_body, mesh=mesh,
                  in_specs=(PartitionSpec("core"),) * (n_params + n_out),
                  out_specs=(PartitionSpec("core"),) * n_out,
                  check_rep=False),
        keep_unused=True,
    )

    zero_outs = [
        jax.device_put(
            np.zeros((n_cores * a.shape[0], *a.shape[1:]), a.dtype),
            NamedSharding(mesh, PartitionSpec("core")),
        )
        for a in out_avals
    ]

    compiled = {"fn": None}

    def run(in_maps):
        concat_in = [
            np.concatenate([np.asarray(m[name]) for m in in_maps], axis=0)
            for name in in_names
        ]
        args = [jax.device_put(a) for a in concat_in]
        if compiled["fn"] is None:
            compiled["fn"] = bass2jax.fast_dispatch_compile(
                lambda: sharded_jit.lower(*args, *zero_outs).compile())
        out_arrs = compiled["fn"](*args, *zero_outs)
        import jax as _jax
        _jax.block_until_ready(out_arrs)
        return [
            {name: np.asarray(out_arrs[i]).reshape(n_cores, *out_avals[i].shape)[c]
             for i, name in enumerate(out_names)}
            for c in range(n_cores)
        ]

    run.in_names = in_names
    run.out_names = out_names
    run.out_avals = out_avals
    run.zero_outs = zero_outs
    run.compiled = compiled
    run.n_cores = n_cores
    _RUNNER_CACHE[gpc] = run
    return run


GPC = 8  # 1 core does the whole problem: per-exec dispatch overhead on the
         # axon path scales with cores x operands and dwarfs device exec
         # (~1.2 ms); measured fastest of the 1/2/8-core variants.


def kernel(x, Wq, bq, Wk, bk, Wv, bv, Wo, bo, **_ignored):
    x = np.asarray(x, dtype=np.float32)
    in_maps = make_in_maps(
        x,
        np.asarray(Wq, np.float32), np.asarray(bq, np.float32),
        np.asarray(Wk, np.float32), np.asarray(bk, np.float32),
        np.asarray(Wv, np.float32), np.asarray(bv, np.float32),
        np.asarray(Wo, np.float32), gpc=GPC,
    )
    try:
        results = get_runner(GPC)(in_maps)
    except Exception:
        # fallback: stock SPMD runner (slower dispatch, same NEFF)
        from concourse.bass_utils import run_bass_kernel_spmd
        results = run_bass_kernel_spmd(
            get_nc(GPC), in_maps, core_ids=list(range(8 // GPC))).results
    return combine_results(results, np.asarray(bo, np.float32), gpc=GPC)
